# revision 14
# baseline (speedup 1.0000x reference)
"""Trainium2 Bass kernel for nn_ConstellationRelay.

Computation (per token, D=1024, A=16 anchors, C=8 comps, dc=64):
  h   = l2norm(layernorm(x; ln_g, ln_b))
  tri = 1 - h @ l2norm(anchors).T                       (N, 16)
  u   = relu(einsum('nak,kae->nke', tri_g, W1) + b1)^2  (N, 8, 128)
  y   = layernorm_c(u @ W2 + b2; cg, cb)                (N, 8, 64)
  out = x + sigmoid(gate) * (y.flat @ Wp + bp)

Strategy: pure data-parallel over batch (one of 8 NeuronCores per batch row).
v3 design:
  * x cast f32->bf16 during the HBM load DMA (SWDGE, one op per tile);
    out written bf16->f32 by the store DMA. No f32 x on chip.
  * stats via tensor_scalar/tensor_tensor accum_out (sum, sumsq) on DVE.
  * residual folded into the proj PSUM group via a 64*I bf16 identity matmul
    (wpf prescaled 2^6 for fp8; drain copies apply 2^-6 -- exact).
  * proj matmul fp8 DoubleRow (contract 2x128 per MM).
  * issue order per round: dma_in(t), back(t-3), mid(t-2), front(t-1) --
    oldest work first in each engine FIFO to avoid head-of-line blocking.
"""

import functools
import os
import sys

import numpy as np

for _p in ("/opt/trn_rl_repo",):
    if _p not in sys.path and os.path.isdir(_p):
        sys.path.insert(0, _p)

B, S, D = 8, 4096, 1024
A, C, DC = 16, 8, 64
APC = A // C  # anchors per compartment
E2 = 2 * DC  # 128, expanded width per comp
NCORES = 8
TOK = 512  # tokens per pipeline tile
NTILE = S // TOK  # 8
NCH = TOK // 128  # 4 token chunks of 128 per tile
KD = D // 128  # 8 feature chunks
PSCALE = 64.0  # 2^6 fp8 pre-scale on wpf (and on the identity residual)


def _np_reference(x, anchors, ln_g, ln_b, W1, b1, W2, b2, cg, cb, Wp, bp, gate):
    x = x.astype(np.float32)
    N = x.shape[0] * x.shape[1]
    xf = x.reshape(N, D)
    mu = xf.mean(-1, keepdims=True)
    var = ((xf - mu) ** 2).mean(-1, keepdims=True)
    h = (xf - mu) / np.sqrt(var + 1e-5) * ln_g + ln_b
    h = h / np.maximum(np.linalg.norm(h, axis=-1, keepdims=True), 1e-12)
    a = anchors / np.maximum(np.linalg.norm(anchors, axis=-1, keepdims=True), 1e-12)
    tri = 1.0 - h @ a.T
    g = tri.reshape(N, APC, C)
    u = np.einsum("nak,kae->nke", g, W1) + b1
    u = np.square(np.maximum(u, 0.0))
    y = np.einsum("nke,ked->nkd", u, W2) + b2
    muy = y.mean(-1, keepdims=True)
    vy = ((y - muy) ** 2).mean(-1, keepdims=True)
    y = (y - muy) / np.sqrt(vy + 1e-5) * cg + cb
    upd = y.reshape(N, C * DC) @ Wp + bp
    sig = 1.0 / (1.0 + np.exp(-gate))
    return (xf + sig * upd).reshape(x.shape).astype(np.float32)


@functools.lru_cache(maxsize=4)
def _build_program(n_tokens=S, use_const=False, interleaved_t=True,
                   use_recip_approx=True, use_fp8=None, use_id=None):
    use_fp8 = USE_FP8_PROJ if use_fp8 is None else use_fp8
    use_id = USE_ID_RESID if use_id is None else use_id
    import concourse.bacc as bacc
    import concourse.mybir as mybir
    import concourse.tile as tile

    f32 = mybir.dt.float32
    bf16 = mybir.dt.bfloat16
    f8 = mybir.dt.float8e4
    AF = mybir.ActivationFunctionType
    OP = mybir.AluOpType
    PM = mybir.MatmulPerfMode

    ntile = n_tokens // TOK

    nc = bacc.Bacc("TRN2", target_bir_lowering=False, debug=False,
                   num_devices=NCORES)

    x_d = nc.dram_tensor("x", [n_tokens, D], f32, kind="ExternalInput")
    agt_d = nc.dram_tensor("agt", [128, KD, 112], bf16, kind="ExternalInput")
    w1e_d = nc.dram_tensor("w1e", [112, KD, 128], bf16, kind="ExternalInput")
    biasu_d = nc.dram_tensor("biasu", [128, KD], f32, kind="ExternalInput")
    w2c_d = nc.dram_tensor("w2c", [128, C, DC], bf16, kind="ExternalInput")
    vstl_d = nc.dram_tensor("vstl", [128, 4, C], bf16, kind="ExternalInput")
    b2f_d = nc.dram_tensor("b2f", [128, 4], f32, kind="ExternalInput")
    if use_fp8:
        wpf_d = nc.dram_tensor("wpf", [128, 2, 2, 2, 512], f8,
                               kind="ExternalInput")
    else:
        wpf_d = nc.dram_tensor("wpf", [128, 4, 2, 512], bf16,
                               kind="ExternalInput")
    sel_d = nc.dram_tensor("sel", [C, 4, 128], bf16, kind="ExternalInput")
    id64_d = nc.dram_tensor("id64", [128, 128], bf16, kind="ExternalInput")
    cvec_d = nc.dram_tensor("cvec", [1, 2, 512], bf16, kind="ExternalInput") \
        if use_const else None
    out_d = nc.dram_tensor("out", [n_tokens, D], f32, kind="ExternalOutput")

    from contextlib import ExitStack

    with tile.TileContext(nc) as tc, ExitStack() as ctx:
        pp = ctx.enter_context(tc.tile_pool(name="params", bufs=1))
        agt = pp.tile([128, KD, 112], bf16)
        nc.sync.dma_start(out=agt, in_=agt_d[:, :, :])
        w1e = pp.tile([112, KD, 128], bf16)
        nc.sync.dma_start(out=w1e, in_=w1e_d[:, :, :])
        biasu = pp.tile([128, KD], f32)
        nc.sync.dma_start(out=biasu, in_=biasu_d[:, :])
        w2c = pp.tile([128, C, DC], bf16)
        nc.sync.dma_start(out=w2c, in_=w2c_d[:, :, :])
        vstl = pp.tile([128, 4, C], bf16)
        nc.sync.dma_start(out=vstl, in_=vstl_d[:, :, :])
        b2f = pp.tile([128, 4], f32)
        nc.sync.dma_start(out=b2f, in_=b2f_d[:, :])
        if use_fp8:
            wpf = pp.tile([128, 2, 2, 2, 512], f8)
            nc.sync.dma_start(out=wpf, in_=wpf_d[:, :, :, :, :])
        else:
            wpf = pp.tile([128, 4, 2, 512], bf16)
            nc.sync.dma_start(out=wpf, in_=wpf_d[:, :, :, :])
        sel = pp.tile([C, 4, 128], bf16)
        nc.sync.dma_start(out=sel, in_=sel_d[:, :, :])
        id64 = pp.tile([128, 128], bf16)
        nc.sync.dma_start(out=id64, in_=id64_d[:, :])
        if use_const:
            cvec = pp.tile([1, 2, 512], bf16)
            nc.sync.dma_start(out=cvec, in_=cvec_d[:, :, :])
            ones1 = pp.tile([1, 128], bf16)
            nc.vector.memset(ones1, 1.0)
        ctiny = pp.tile([128, 1], f32)
        nc.vector.memset(ctiny, 1e-38)
        ceps = pp.tile([C, 1], f32)
        nc.vector.memset(ceps, 1e-5)

        px = ctx.enter_context(tc.tile_pool(name="px", bufs=2))
        psm = ctx.enter_context(tc.tile_pool(name="psm", bufs=8))
        # PSUM pools: 4 + 2 + 2 = 8 banks exactly.
        ps_small = ctx.enter_context(tc.tile_pool(name="ps_small", bufs=2,
                                                  space="PSUM"))
        ps_y = ctx.enter_context(tc.tile_pool(name="ps_y", bufs=2,
                                              space="PSUM"))
        ps_mm = ctx.enter_context(tc.tile_pool(name="ps_mm", bufs=4,
                                               space="PSUM"))

        def stage_load(t):
            row0 = t * TOK
            xb = px.tile([128, NCH, D], bf16, tag="xb", bufs=4, name=f"xb{t}")
            nc.gpsimd.dma_start(
                out=xb,
                in_=x_d[row0:row0 + TOK, :].rearrange(
                    "(c p) d -> p c d", p=128))
            return xb

        def stage_front(t, xb):
            """Stats + normalize + transpose."""
            scr = px.tile([128, 2, D], bf16, tag="scr", bufs=2)
            sums = psm.tile([128, NCH], f32, tag="sums")
            sumsq = psm.tile([128, NCH], f32, tag="sumsq")
            for cch in range(NCH):
                nc.vector.tensor_scalar(
                    out=scr[:, 0, :], in0=xb[:, cch, :], scalar1=1.0,
                    scalar2=0.0, op0=OP.mult, op1=OP.add,
                    accum_out=sums[:, cch:cch + 1])
                nc.vector.scalar_tensor_tensor(
                    out=scr[:, 1, :], in0=xb[:, cch, :], scalar=1.0,
                    in1=xb[:, cch, :], op0=OP.mult, op1=OP.mult,
                    accum_out=sumsq[:, cch:cch + 1])
            negq = psm.tile([128, NCH], f32, tag="negq")
            nc.vector.scalar_tensor_tensor(
                out=negq, in0=sums, scalar=-1.0 / D, in1=sums,
                op0=OP.mult, op1=OP.mult)
            dvar = psm.tile([128, NCH], f32, tag="dvar")
            nc.vector.tensor_add(dvar, sumsq, negq)
            sd = psm.tile([128, NCH], f32, tag="sd")
            nc.scalar.activation(sd, dvar, AF.Sqrt, bias=ctiny, scale=1.0)
            ee = psm.tile([128, NCH], f32, tag="ee", name=f"ee{t}")
            nc.vector.reciprocal(ee, sd)
            bh = psm.tile([128, NCH], f32, tag="bh", name=f"bh{t}")
            nc.vector.scalar_tensor_tensor(
                out=bh, in0=sums, scalar=-1.0 / D, in1=ee,
                op0=OP.mult, op1=OP.mult)
            hb = px.tile([128, NCH, D], bf16, tag="hb", bufs=2, name=f"hb{t}")
            for cch in range(NCH):
                nc.vector.tensor_scalar(
                    out=hb[:, cch, :], in0=xb[:, cch, :],
                    scalar1=ee[:, cch:cch + 1], scalar2=bh[:, cch:cch + 1],
                    op0=OP.mult, op1=OP.add)
            hbT = px.tile([128, KD, TOK], bf16, tag="hbT", bufs=3,
                          name=f"hbT{t}")
            for cch in range(NCH):
                nc.sync.dma_start_transpose(
                    out=hbT[:, :, cch * 128:(cch + 1) * 128],
                    in_=hb[:, cch, :])
            return hbT

        def stage_mid(t, xb, hbT):
            # --- A0 = a_norm @ h, 4 replicas at partitions {0,32,64,96} ---
            a0p = ps_small.tile([112, TOK], f32, tag="small")
            for dch in range(KD):
                nc.tensor.matmul(a0p, lhsT=agt[:, dch, :], rhs=hbT[:, dch, :],
                                 start=(dch == 0), stop=(dch == KD - 1))
            a0 = psm.tile([112, TOK], bf16, tag="a0", bufs=2)
            nc.scalar.copy(out=a0, in_=a0p)

            # --- expand (4-way row-packed) + relu; square on GPS ----------
            rbig = px.tile([128, KD, TOK], bf16, tag="rbig", bufs=2)
            ubig = px.tile([128, KD, TOK], bf16, tag="ubig", bufs=2)
            for kg in range(2):
                ups = []
                for r in range(4):
                    k = 4 * kg + r
                    up = ps_mm.tile([128, TOK], f32, tag="mmout")
                    nc.tensor.matmul(
                        up, lhsT=w1e[32 * r:32 * r + A, k, :],
                        rhs=a0[32 * r:32 * r + A, :],
                        start=True, stop=True,
                        tile_position=(32 * r, 0))
                    ups.append(up)
                for r in range(4):
                    k = 4 * kg + r
                    nc.scalar.activation(rbig[:, k, :], ups[r], AF.Relu,
                                         bias=biasu[:, k:k + 1], scale=1.0)
                    nc.gpsimd.tensor_mul(ubig[:, k, :], rbig[:, k, :],
                                         rbig[:, k, :])

            # --- comp matmul; yb via DVE ts, sqy via ACT Square -----------
            yb = px.tile([128, 4, TOK], bf16, tag="yb", bufs=3,
                         name=f"yb{t}")
            sqy = px.tile([128, 4, TOK], bf16, tag="sqy", bufs=2)
            for j in range(4):
                yp = ps_y.tile([128, TOK], f32, tag="ypre")
                nc.tensor.matmul(yp[0:64, :], lhsT=w2c[:, 2 * j, :],
                                 rhs=ubig[:, 2 * j, :], start=True, stop=True)
                nc.tensor.matmul(yp[64:128, :], lhsT=w2c[:, 2 * j + 1, :],
                                 rhs=ubig[:, 2 * j + 1, :], start=True,
                                 stop=True, tile_position=(0, 64))
                nc.vector.tensor_scalar(
                    out=yb[:, j, :], in0=yp, scalar1=b2f[:, j:j + 1],
                    scalar2=None, op0=OP.add)
                nc.scalar.activation(sqy[:, j, :], yp, AF.Square,
                                     bias=b2f[:, j:j + 1], scale=1.0)

            # --- per-comp variance via PE; rstd = 1/sqrt(var+eps) ---------
            vst = ps_small.tile([C, TOK], f32, tag="small")
            for j in range(4):
                nc.tensor.matmul(vst, lhsT=vstl[:, j, :], rhs=sqy[:, j, :],
                                 start=(j == 0), stop=(j == 3))
            sd2 = psm.tile([C, TOK], f32, tag="sd2", bufs=2)
            nc.scalar.activation(sd2, vst, AF.Sqrt, bias=ceps, scale=1.0)
            rr = psm.tile([C, TOK], f32, tag="rr", bufs=2)
            if use_recip_approx:
                nc.vector.reciprocal_approx_fast(out=rr, in_=sd2)
            else:
                nc.vector.reciprocal(out=rr, in_=sd2)
            rrb = psm.tile([C, TOK], bf16, tag="rrb", bufs=3, name=f"rrb{t}")
            nc.vector.tensor_copy(out=rrb, in_=rr)
            return yb, rrb

        def stage_back(t, xb, yb, rrb):
            row0 = t * TOK
            # rstd broadcast via selector matmuls; ycT = yb * rstd (fp8)
            ycT = px.tile([128, 4, TOK], f8 if use_fp8 else bf16,
                          tag="ycT", bufs=2)
            for j in range(4):
                rbP = ps_mm.tile([128, TOK], f32, tag="mmout")
                nc.tensor.matmul(rbP, lhsT=sel[:, j, :], rhs=rrb,
                                 start=True, stop=True)
                nc.vector.tensor_mul(ycT[:, j, :], yb[:, j, :], rbP)

            # --- proj (fp8 DoubleRow) + identity residual in PSUM ---------
            osb = px.tile([128, NCH, D], bf16, tag="osb", bufs=2)
            for cch in range(NCH):
                for hf in range(2):
                    ud = ps_mm.tile([128, 512], f32, tag="mmout")
                    # last matmul of the accumulation group carries stop=True
                    last_is_proj = not (use_const or use_id)
                    if use_fp8:
                        for g in range(2):
                            nc.tensor.matmul(
                                ud,
                                lhsT=ycT[:, 2 * g:2 * g + 2,
                                         cch * 128:(cch + 1) * 128],
                                rhs=wpf[:, g, :, hf, :],
                                start=(g == 0),
                                stop=(last_is_proj and g == 1),
                                perf_mode=PM.DoubleRow)
                    else:
                        for j in range(4):
                            nc.tensor.matmul(
                                ud,
                                lhsT=ycT[:, j, cch * 128:(cch + 1) * 128],
                                rhs=wpf[:, j, hf, :],
                                start=(j == 0),
                                stop=(last_is_proj and j == 3))
                    if use_const:
                        nc.tensor.matmul(ud, lhsT=ones1, rhs=cvec[:, hf, :],
                                         start=False, stop=not use_id)
                    dst = osb[:, cch, hf * 512:(hf + 1) * 512]
                    if use_id:
                        nc.tensor.matmul(
                            ud, lhsT=id64,
                            rhs=xb[:, cch, hf * 512:(hf + 1) * 512],
                            start=False, stop=True)
                        if hf == 0:
                            nc.vector.tensor_scalar(
                                out=dst, in0=ud, scalar1=1.0 / PSCALE,
                                scalar2=None, op0=OP.mult)
                        else:
                            nc.scalar.activation(dst, ud, AF.Copy,
                                                 scale=1.0 / PSCALE)
                    else:
                        nc.vector.scalar_tensor_tensor(
                            out=dst, in0=ud, scalar=1.0 / PSCALE,
                            in1=xb[:, cch, hf * 512:(hf + 1) * 512],
                            op0=OP.mult, op1=OP.add)
            nc.gpsimd.dma_start(
                out=out_d[row0:row0 + TOK, :].rearrange(
                    "(c p) d -> p c d", p=128),
                in_=osb)

        # Software pipeline: load(t) | front(t-1) | mid(t-2) | back(t-3).
        # Issue order within a round: prefetch DMA first, then oldest work.
        xbs, hbTs, mds = {}, {}, {}
        for t in range(ntile + 3):
            if t < ntile:
                xbs[t] = stage_load(t)
            if t >= 3:
                yb_, rrb_ = mds.pop(t - 3)
                stage_back(t - 3, xbs.pop(t - 3), yb_, rrb_)
            if 2 <= t <= ntile + 1:
                mds[t - 2] = stage_mid(t - 2, xbs[t - 2], hbTs.pop(t - 2))
            if 1 <= t <= ntile:
                hbTs[t - 1] = stage_front(t - 1, xbs[t - 1])

    nc.compile()
    return nc


def _pack_params(anchors, ln_g, W1, b1, W2, b2, cg, cb, Wp, bp, gate,
                 interleaved_t=True):
    f32 = np.float32
    anchors = anchors.astype(f32)
    an = anchors / np.maximum(
        np.linalg.norm(anchors.astype(np.float64), axis=1, keepdims=True),
        1e-12).astype(f32)
    ag = (an * ln_g[None, :].astype(f32)).astype(f32)  # [A, D]

    # agt[p, s, 32r+m] = ag[m, d(p,s)] for r in 0..3 (4 replicas)
    agt = np.zeros((128, KD, 112), f32)
    dd = np.arange(D)
    if interleaved_t:
        pidx, sidx = dd // KD, dd % KD
    else:
        pidx, sidx = dd % 128, dd // 128
    for r in range(4):
        agt[pidx, sidx, 32 * r:32 * r + A] = ag.T[dd, :]

    # W1exp[m, f] with m=j*C+k2, f=k*128+e -> value W1[k, j, e] iff k2==k
    W1 = W1.astype(f32)
    w1exp = np.zeros((A, C, E2), f32)
    for m in range(A):
        j, k2 = m // C, m % C
        w1exp[m, k2, :] = W1[k2, j, :]
    w1e16 = (-w1exp).reshape(A, C, E2)  # [16, 8, 128] (f = k*128+e)
    w1e = np.zeros((112, C, E2), f32)
    for r in range(4):
        w1e[32 * r:32 * r + A] = w1e16
    sf = w1exp.sum(axis=0)  # [C, E2]
    biasu = (sf + b1.astype(f32)).T.copy()  # [128, C] (partition=e, col=k)

    W2 = W2.astype(f32)
    w2m = W2.mean(axis=2, keepdims=True)
    w2cent = W2 - w2m  # [C, E2, DC]
    w2c = np.transpose(w2cent, (1, 0, 2)).copy()  # [128, C, 64]
    b2c = b2.astype(f32) - b2.astype(f32).mean(axis=1, keepdims=True)  # [C, DC]

    b2f = np.zeros((128, 4), f32)
    vstl = np.zeros((128, 4, C), f32)
    for j in range(4):
        for p in range(128):
            kk = 2 * j + p // 64
            b2f[p, j] = b2c[kk, p % 64]
            vstl[p, j, kk] = 1.0 / DC

    sig = (1.0 / (1.0 + np.exp(-gate.astype(np.float64)))).astype(f32)  # [D]
    wpfold = (cg.astype(f32).reshape(C * DC, 1) * Wp.astype(f32)) * sig[None, :]
    wpfold = wpfold * PSCALE
    if USE_FP8_PROJ:
        # DoubleRow: wpf[p, g, i, hf, f] = wpfold[(2g+i)*128 + p, hf*512+f]
        wpf = np.ascontiguousarray(
            wpfold.reshape(2, 2, 128, 2, 512).transpose(2, 0, 1, 3, 4))
    else:
        wpf = np.ascontiguousarray(
            wpfold.reshape(4, 128, 2, 512).transpose(1, 0, 2, 3))

    const = (cb.astype(f32).reshape(-1) @ Wp.astype(f32) + bp.astype(f32)) * sig
    use_const = bool(np.max(np.abs(const)) > 0)

    import ml_dtypes
    bf16 = ml_dtypes.bfloat16
    import concourse.mybir as mybir
    f8np = mybir.dt.np(mybir.dt.float8e4)
    sel = np.zeros((C, 4, 128), f32)
    for j in range(4):
        sel[2 * j, j, 0:64] = 1.0
        sel[2 * j + 1, j, 64:128] = 1.0

    params = dict(
        sel=sel.astype(bf16),
        agt=agt.astype(bf16),
        w1e=w1e.astype(bf16),
        biasu=biasu.astype(f32),
        w2c=w2c.astype(bf16),
        vstl=vstl.astype(bf16),
        b2f=b2f.astype(f32),
        wpf=wpf.astype(f8np if USE_FP8_PROJ else bf16),
        id64=(np.eye(128, dtype=f32) * PSCALE).astype(bf16),
    )
    if use_const:
        params["cvec"] = const.reshape(1, 2, 512).astype(bf16)
    return params, use_const


def kernel(**inputs):
    x = np.asarray(inputs["x"], dtype=np.float32)
    ln_g = np.asarray(inputs["ln_g"], dtype=np.float32)
    ln_b = np.asarray(inputs["ln_b"], dtype=np.float32)

    fast = (np.allclose(ln_g, 1.0, atol=1e-12) and
            np.allclose(ln_b, 0.0, atol=1e-12))
    if not fast:
        return _np_reference(
            x, *[np.asarray(inputs[k], dtype=np.float32) for k in
                 ("anchors", "ln_g", "ln_b", "W1", "b1", "W2", "b2", "cg",
                  "cb", "Wp", "bp", "gate")])

    params, use_const = _pack_params(
        inputs["anchors"], ln_g, inputs["W1"], inputs["b1"], inputs["W2"],
        inputs["b2"], inputs["cg"], inputs["cb"], inputs["Wp"], inputs["bp"],
        inputs["gate"], interleaved_t=INTERLEAVED_T)

    nc = _build_program(S, use_const, INTERLEAVED_T, USE_RECIP_APPROX)

    from concourse.bass_utils import run_bass_kernel_spmd
    in_maps = []
    for b in range(NCORES):
        m = dict(params)
        m["x"] = np.ascontiguousarray(x[b])
        in_maps.append(m)
    res = run_bass_kernel_spmd(nc, in_maps, core_ids=list(range(NCORES)))
    out = np.stack([res.results[b]["out"] for b in range(NCORES)], axis=0)
    return out.reshape(B, S, D).astype(np.float32)


INTERLEAVED_T = True
USE_RECIP_APPROX = True
USE_FP8_PROJ = False
USE_ID_RESID = True


# revision 16
# speedup vs baseline: 1.0900x; 1.0900x over previous
"""Trainium2 Bass kernel for nn_ConstellationRelay.

Computation (per token, D=1024, A=16 anchors, C=8 comps, dc=64):
  h   = l2norm(layernorm(x; ln_g, ln_b))
  tri = 1 - h @ l2norm(anchors).T                       (N, 16)
  u   = relu(einsum('nak,kae->nke', tri_g, W1) + b1)^2  (N, 8, 128)
  y   = layernorm_c(u @ W2 + b2; cg, cb)                (N, 8, 64)
  out = x + sigmoid(gate) * (y.flat @ Wp + bp)

Strategy: pure data-parallel over batch (one of 8 NeuronCores per batch row).
v3 design:
  * x cast f32->bf16 during the HBM load DMA (SWDGE, one op per tile);
    out written bf16->f32 by the store DMA. No f32 x on chip.
  * stats via tensor_scalar/tensor_tensor accum_out (sum, sumsq) on DVE.
  * residual folded into the proj PSUM group via a 64*I bf16 identity matmul
    (wpf prescaled 2^6 for fp8; drain copies apply 2^-6 -- exact).
  * proj matmul fp8 DoubleRow (contract 2x128 per MM).
  * issue order per round: dma_in(t), back(t-3), mid(t-2), front(t-1) --
    oldest work first in each engine FIFO to avoid head-of-line blocking.
"""

import functools
import os
import sys

import numpy as np

for _p in ("/opt/trn_rl_repo",):
    if _p not in sys.path and os.path.isdir(_p):
        sys.path.insert(0, _p)

B, S, D = 8, 4096, 1024
A, C, DC = 16, 8, 64
APC = A // C  # anchors per compartment
E2 = 2 * DC  # 128, expanded width per comp
NCORES = 8
TOK = 512  # tokens per pipeline tile
NTILE = S // TOK  # 8
NCH = TOK // 128  # 4 token chunks of 128 per tile
KD = D // 128  # 8 feature chunks
PSCALE = 64.0  # 2^6 fp8 pre-scale on wpf (and on the identity residual)


def _np_reference(x, anchors, ln_g, ln_b, W1, b1, W2, b2, cg, cb, Wp, bp, gate):
    x = x.astype(np.float32)
    N = x.shape[0] * x.shape[1]
    xf = x.reshape(N, D)
    mu = xf.mean(-1, keepdims=True)
    var = ((xf - mu) ** 2).mean(-1, keepdims=True)
    h = (xf - mu) / np.sqrt(var + 1e-5) * ln_g + ln_b
    h = h / np.maximum(np.linalg.norm(h, axis=-1, keepdims=True), 1e-12)
    a = anchors / np.maximum(np.linalg.norm(anchors, axis=-1, keepdims=True), 1e-12)
    tri = 1.0 - h @ a.T
    g = tri.reshape(N, APC, C)
    u = np.einsum("nak,kae->nke", g, W1) + b1
    u = np.square(np.maximum(u, 0.0))
    y = np.einsum("nke,ked->nkd", u, W2) + b2
    muy = y.mean(-1, keepdims=True)
    vy = ((y - muy) ** 2).mean(-1, keepdims=True)
    y = (y - muy) / np.sqrt(vy + 1e-5) * cg + cb
    upd = y.reshape(N, C * DC) @ Wp + bp
    sig = 1.0 / (1.0 + np.exp(-gate))
    return (xf + sig * upd).reshape(x.shape).astype(np.float32)


@functools.lru_cache(maxsize=4)
def _build_program(n_tokens=S, use_const=False, interleaved_t=True,
                   use_recip_approx=True, use_fp8=None, use_id=None):
    use_fp8 = USE_FP8_PROJ if use_fp8 is None else use_fp8
    use_id = USE_ID_RESID if use_id is None else use_id
    import concourse.bacc as bacc
    import concourse.mybir as mybir
    import concourse.tile as tile

    f32 = mybir.dt.float32
    bf16 = mybir.dt.bfloat16
    f8 = mybir.dt.float8e4
    AF = mybir.ActivationFunctionType
    OP = mybir.AluOpType
    PM = mybir.MatmulPerfMode

    ntile = n_tokens // TOK

    nc = bacc.Bacc("TRN2", target_bir_lowering=False, debug=False,
                   num_devices=NCORES)

    x_d = nc.dram_tensor("x", [n_tokens, D], f32, kind="ExternalInput")
    agt_d = nc.dram_tensor("agt", [128, KD, 112], bf16, kind="ExternalInput")
    w1e_d = nc.dram_tensor("w1e", [112, KD, 128], bf16, kind="ExternalInput")
    biasu_d = nc.dram_tensor("biasu", [128, KD], f32, kind="ExternalInput")
    w2c_d = nc.dram_tensor("w2c", [128, C, DC], bf16, kind="ExternalInput")
    vstl_d = nc.dram_tensor("vstl", [128, 4, C], bf16, kind="ExternalInput")
    b2f_d = nc.dram_tensor("b2f", [128, 4], f32, kind="ExternalInput")
    if use_fp8:
        wpf_d = nc.dram_tensor("wpf", [128, 2, 2, 2, 512], f8,
                               kind="ExternalInput")
    else:
        wpf_d = nc.dram_tensor("wpf", [128, 4, 2, 512], bf16,
                               kind="ExternalInput")
    sel_d = nc.dram_tensor("sel", [C, 4, 128], bf16, kind="ExternalInput")
    id64_d = nc.dram_tensor("id64", [128, 128], bf16, kind="ExternalInput")
    cvec_d = nc.dram_tensor("cvec", [1, 2, 512], bf16, kind="ExternalInput") \
        if use_const else None
    out_d = nc.dram_tensor("out", [n_tokens, D], f32, kind="ExternalOutput")

    from contextlib import ExitStack

    with tile.TileContext(nc) as tc, ExitStack() as ctx:
        pp = ctx.enter_context(tc.tile_pool(name="params", bufs=1))
        agt = pp.tile([128, KD, 112], bf16)
        nc.sync.dma_start(out=agt, in_=agt_d[:, :, :])
        w1e = pp.tile([112, KD, 128], bf16)
        nc.sync.dma_start(out=w1e, in_=w1e_d[:, :, :])
        biasu = pp.tile([128, KD], f32)
        nc.sync.dma_start(out=biasu, in_=biasu_d[:, :])
        w2c = pp.tile([128, C, DC], bf16)
        nc.sync.dma_start(out=w2c, in_=w2c_d[:, :, :])
        vstl = pp.tile([128, 4, C], bf16)
        nc.sync.dma_start(out=vstl, in_=vstl_d[:, :, :])
        b2f = pp.tile([128, 4], f32)
        nc.sync.dma_start(out=b2f, in_=b2f_d[:, :])
        if use_fp8:
            wpf = pp.tile([128, 2, 2, 2, 512], f8)
            nc.sync.dma_start(out=wpf, in_=wpf_d[:, :, :, :, :])
        else:
            wpf = pp.tile([128, 4, 2, 512], bf16)
            nc.sync.dma_start(out=wpf, in_=wpf_d[:, :, :, :])
        sel = pp.tile([C, 4, 128], bf16)
        nc.sync.dma_start(out=sel, in_=sel_d[:, :, :])
        id64 = pp.tile([128, 128], bf16)
        nc.sync.dma_start(out=id64, in_=id64_d[:, :])
        if use_const:
            cvec = pp.tile([1, 2, 512], bf16)
            nc.sync.dma_start(out=cvec, in_=cvec_d[:, :, :])
            ones1 = pp.tile([1, 128], bf16)
            nc.vector.memset(ones1, 1.0)
        ctiny = pp.tile([128, 1], f32)
        nc.vector.memset(ctiny, 1e-38)
        ceps = pp.tile([C, 1], f32)
        nc.vector.memset(ceps, 1e-5)

        px = ctx.enter_context(tc.tile_pool(name="px", bufs=2))
        psm = ctx.enter_context(tc.tile_pool(name="psm", bufs=8))
        # PSUM pools: 4 + 2 + 2 = 8 banks exactly.
        ps_small = ctx.enter_context(tc.tile_pool(name="ps_small", bufs=2,
                                                  space="PSUM"))
        ps_y = ctx.enter_context(tc.tile_pool(name="ps_y", bufs=2,
                                              space="PSUM"))
        ps_mm = ctx.enter_context(tc.tile_pool(name="ps_mm", bufs=4,
                                               space="PSUM"))

        def stage_load(t):
            row0 = t * TOK
            xb = px.tile([128, NCH, D], bf16, tag="xb", bufs=4, name=f"xb{t}")
            nc.gpsimd.dma_start(
                out=xb,
                in_=x_d[row0:row0 + TOK, :].rearrange(
                    "(c p) d -> p c d", p=128))
            return xb

        def stage_front(t, xb):
            """Stats + normalize + transpose."""
            mv = psm.tile([128, NCH, 2], f32, tag="mv", name=f"mv{t}")
            for cch in range(NCH):
                st = psm.tile([128, 2, 6], f32, tag="st")
                xr = xb[:, cch, :].rearrange("p (s f) -> p s f", s=2)
                nc.vector.bn_stats(out=st[:, 0, :], in_=xr[:, 0, :])
                nc.vector.bn_stats(out=st[:, 1, :], in_=xr[:, 1, :])
                nc.vector.bn_aggr(out=mv[:, cch, :], in_=st)
            sd = psm.tile([128, NCH], f32, tag="sd")
            nc.scalar.activation(sd, mv[:, :, 1], AF.Sqrt, bias=ctiny,
                                 scale=float(D))
            ee = psm.tile([128, NCH], f32, tag="ee", name=f"ee{t}")
            nc.vector.reciprocal(ee, sd)
            bh = psm.tile([128, NCH], f32, tag="bh", name=f"bh{t}")
            nc.vector.scalar_tensor_tensor(
                out=bh, in0=mv[:, :, 0], scalar=-1.0, in1=ee,
                op0=OP.mult, op1=OP.mult)
            hb = px.tile([128, NCH, D], bf16, tag="hb", bufs=3, name=f"hb{t}")
            for cch in range(NCH):
                nc.vector.tensor_scalar(
                    out=hb[:, cch, :], in0=xb[:, cch, :],
                    scalar1=ee[:, cch:cch + 1], scalar2=bh[:, cch:cch + 1],
                    op0=OP.mult, op1=OP.add)
            hbT = px.tile([128, KD, TOK], bf16, tag="hbT", bufs=3,
                          name=f"hbT{t}")
            for cch in range(NCH):
                nc.sync.dma_start_transpose(
                    out=hbT[:, :, cch * 128:(cch + 1) * 128],
                    in_=hb[:, cch, :])
            return hbT

        def stage_mid(t, xb, hbT):
            # --- A0 = a_norm @ h, 4 replicas at partitions {0,32,64,96} ---
            a0p = ps_small.tile([112, TOK], f32, tag="small")
            for dch in range(KD):
                nc.tensor.matmul(a0p, lhsT=agt[:, dch, :], rhs=hbT[:, dch, :],
                                 start=(dch == 0), stop=(dch == KD - 1))
            a0 = psm.tile([112, TOK], bf16, tag="a0", bufs=3)
            nc.scalar.copy(out=a0, in_=a0p)

            # --- expand (4-way row-packed) + relu; square on GPS ----------
            rbig = px.tile([128, KD, TOK], bf16, tag="rbig", bufs=3)
            ubig = px.tile([128, KD, TOK], bf16, tag="ubig", bufs=3)
            for kg in range(2):
                ups = []
                for r in range(4):
                    k = 4 * kg + r
                    up = ps_mm.tile([128, TOK], f32, tag="mmout")
                    nc.tensor.matmul(
                        up, lhsT=w1e[32 * r:32 * r + A, k, :],
                        rhs=a0[32 * r:32 * r + A, :],
                        start=True, stop=True,
                        tile_position=(32 * r, 0))
                    ups.append(up)
                for r in range(4):
                    k = 4 * kg + r
                    nc.scalar.activation(rbig[:, k, :], ups[r], AF.Relu,
                                         bias=biasu[:, k:k + 1], scale=1.0)
                    nc.gpsimd.tensor_mul(ubig[:, k, :], rbig[:, k, :],
                                         rbig[:, k, :])

            # --- comp matmul; yb via DVE ts, sqy via ACT Square -----------
            yb = px.tile([128, 4, TOK], bf16, tag="yb", bufs=3,
                         name=f"yb{t}")
            sqy = px.tile([128, 4, TOK], bf16, tag="sqy", bufs=3)
            for j in range(4):
                yp = ps_y.tile([128, TOK], f32, tag="ypre")
                nc.tensor.matmul(yp[0:64, :], lhsT=w2c[:, 2 * j, :],
                                 rhs=ubig[:, 2 * j, :], start=True, stop=True)
                nc.tensor.matmul(yp[64:128, :], lhsT=w2c[:, 2 * j + 1, :],
                                 rhs=ubig[:, 2 * j + 1, :], start=True,
                                 stop=True, tile_position=(0, 64))
                nc.vector.tensor_scalar(
                    out=yb[:, j, :], in0=yp, scalar1=b2f[:, j:j + 1],
                    scalar2=None, op0=OP.add)
                nc.scalar.activation(sqy[:, j, :], yp, AF.Square,
                                     bias=b2f[:, j:j + 1], scale=1.0)

            # --- per-comp variance via PE; rstd = 1/sqrt(var+eps) ---------
            vst = ps_small.tile([C, TOK], f32, tag="small")
            for j in range(4):
                nc.tensor.matmul(vst, lhsT=vstl[:, j, :], rhs=sqy[:, j, :],
                                 start=(j == 0), stop=(j == 3))
            sd2 = psm.tile([C, TOK], f32, tag="sd2", bufs=2)
            nc.scalar.activation(sd2, vst, AF.Sqrt, bias=ceps, scale=1.0)
            rr = psm.tile([C, TOK], f32, tag="rr", bufs=2)
            if use_recip_approx:
                nc.vector.reciprocal_approx_fast(out=rr, in_=sd2)
            else:
                nc.vector.reciprocal(out=rr, in_=sd2)
            rrb = psm.tile([C, TOK], bf16, tag="rrb", bufs=3, name=f"rrb{t}")
            nc.vector.tensor_copy(out=rrb, in_=rr)
            return yb, rrb

        def stage_back(t, xb, yb, rrb):
            row0 = t * TOK
            # rstd broadcast via selector matmuls; ycT = yb * rstd (fp8)
            ycT = px.tile([128, 4, TOK], f8 if use_fp8 else bf16,
                          tag="ycT", bufs=2)
            for j in range(4):
                rbP = ps_mm.tile([128, TOK], f32, tag="mmout")
                nc.tensor.matmul(rbP, lhsT=sel[:, j, :], rhs=rrb,
                                 start=True, stop=True)
                nc.vector.tensor_mul(ycT[:, j, :], yb[:, j, :], rbP)

            # --- proj (fp8 DoubleRow) + identity residual in PSUM ---------
            osb = px.tile([128, NCH, D], bf16, tag="osb", bufs=3)
            for cch in range(NCH):
                for hf in range(2):
                    ud = ps_mm.tile([128, 512], f32, tag="mmout")
                    # last matmul of the accumulation group carries stop=True
                    last_is_proj = not (use_const or use_id)
                    if use_fp8:
                        for g in range(2):
                            nc.tensor.matmul(
                                ud,
                                lhsT=ycT[:, 2 * g:2 * g + 2,
                                         cch * 128:(cch + 1) * 128],
                                rhs=wpf[:, g, :, hf, :],
                                start=(g == 0),
                                stop=(last_is_proj and g == 1),
                                perf_mode=PM.DoubleRow)
                    else:
                        for j in range(4):
                            nc.tensor.matmul(
                                ud,
                                lhsT=ycT[:, j, cch * 128:(cch + 1) * 128],
                                rhs=wpf[:, j, hf, :],
                                start=(j == 0),
                                stop=(last_is_proj and j == 3))
                    if use_const:
                        nc.tensor.matmul(ud, lhsT=ones1, rhs=cvec[:, hf, :],
                                         start=False, stop=not use_id)
                    dst = osb[:, cch, hf * 512:(hf + 1) * 512]
                    if use_id:
                        nc.tensor.matmul(
                            ud, lhsT=id64,
                            rhs=xb[:, cch, hf * 512:(hf + 1) * 512],
                            start=False, stop=True)
                        if hf == 0:
                            nc.vector.tensor_scalar(
                                out=dst, in0=ud, scalar1=1.0 / PSCALE,
                                scalar2=None, op0=OP.mult)
                        else:
                            nc.scalar.activation(dst, ud, AF.Copy,
                                                 scale=1.0 / PSCALE)
                    else:
                        nc.vector.scalar_tensor_tensor(
                            out=dst, in0=ud, scalar=1.0 / PSCALE,
                            in1=xb[:, cch, hf * 512:(hf + 1) * 512],
                            op0=OP.mult, op1=OP.add)
            nc.gpsimd.dma_start(
                out=out_d[row0:row0 + TOK, :].rearrange(
                    "(c p) d -> p c d", p=128),
                in_=osb)

        # Software pipeline: load(t) | front(t-1) | mid(t-2) | back(t-3).
        # Issue order within a round: prefetch DMA first, then oldest work.
        xbs, hbTs, mds = {}, {}, {}
        for t in range(ntile + 3):
            if t < ntile:
                xbs[t] = stage_load(t)
            if t >= 3:
                yb_, rrb_ = mds.pop(t - 3)
                stage_back(t - 3, xbs.pop(t - 3), yb_, rrb_)
            if 2 <= t <= ntile + 1:
                mds[t - 2] = stage_mid(t - 2, xbs[t - 2], hbTs.pop(t - 2))
            if 1 <= t <= ntile:
                hbTs[t - 1] = stage_front(t - 1, xbs[t - 1])

    nc.compile()
    return nc


def _pack_params(anchors, ln_g, W1, b1, W2, b2, cg, cb, Wp, bp, gate,
                 interleaved_t=True):
    f32 = np.float32
    anchors = anchors.astype(f32)
    an = anchors / np.maximum(
        np.linalg.norm(anchors.astype(np.float64), axis=1, keepdims=True),
        1e-12).astype(f32)
    ag = (an * ln_g[None, :].astype(f32)).astype(f32)  # [A, D]

    # agt[p, s, 32r+m] = ag[m, d(p,s)] for r in 0..3 (4 replicas)
    agt = np.zeros((128, KD, 112), f32)
    dd = np.arange(D)
    if interleaved_t:
        pidx, sidx = dd // KD, dd % KD
    else:
        pidx, sidx = dd % 128, dd // 128
    for r in range(4):
        agt[pidx, sidx, 32 * r:32 * r + A] = ag.T[dd, :]

    # W1exp[m, f] with m=j*C+k2, f=k*128+e -> value W1[k, j, e] iff k2==k
    W1 = W1.astype(f32)
    w1exp = np.zeros((A, C, E2), f32)
    for m in range(A):
        j, k2 = m // C, m % C
        w1exp[m, k2, :] = W1[k2, j, :]
    w1e16 = (-w1exp).reshape(A, C, E2)  # [16, 8, 128] (f = k*128+e)
    w1e = np.zeros((112, C, E2), f32)
    for r in range(4):
        w1e[32 * r:32 * r + A] = w1e16
    sf = w1exp.sum(axis=0)  # [C, E2]
    biasu = (sf + b1.astype(f32)).T.copy()  # [128, C] (partition=e, col=k)

    W2 = W2.astype(f32)
    w2m = W2.mean(axis=2, keepdims=True)
    w2cent = W2 - w2m  # [C, E2, DC]
    w2c = np.transpose(w2cent, (1, 0, 2)).copy()  # [128, C, 64]
    b2c = b2.astype(f32) - b2.astype(f32).mean(axis=1, keepdims=True)  # [C, DC]

    b2f = np.zeros((128, 4), f32)
    vstl = np.zeros((128, 4, C), f32)
    for j in range(4):
        for p in range(128):
            kk = 2 * j + p // 64
            b2f[p, j] = b2c[kk, p % 64]
            vstl[p, j, kk] = 1.0 / DC

    sig = (1.0 / (1.0 + np.exp(-gate.astype(np.float64)))).astype(f32)  # [D]
    wpfold = (cg.astype(f32).reshape(C * DC, 1) * Wp.astype(f32)) * sig[None, :]
    wpfold = wpfold * PSCALE
    if USE_FP8_PROJ:
        # DoubleRow: wpf[p, g, i, hf, f] = wpfold[(2g+i)*128 + p, hf*512+f]
        wpf = np.ascontiguousarray(
            wpfold.reshape(2, 2, 128, 2, 512).transpose(2, 0, 1, 3, 4))
    else:
        wpf = np.ascontiguousarray(
            wpfold.reshape(4, 128, 2, 512).transpose(1, 0, 2, 3))

    const = (cb.astype(f32).reshape(-1) @ Wp.astype(f32) + bp.astype(f32)) * sig
    use_const = bool(np.max(np.abs(const)) > 0)

    import ml_dtypes
    bf16 = ml_dtypes.bfloat16
    import concourse.mybir as mybir
    f8np = mybir.dt.np(mybir.dt.float8e4)
    sel = np.zeros((C, 4, 128), f32)
    for j in range(4):
        sel[2 * j, j, 0:64] = 1.0
        sel[2 * j + 1, j, 64:128] = 1.0

    params = dict(
        sel=sel.astype(bf16),
        agt=agt.astype(bf16),
        w1e=w1e.astype(bf16),
        biasu=biasu.astype(f32),
        w2c=w2c.astype(bf16),
        vstl=vstl.astype(bf16),
        b2f=b2f.astype(f32),
        wpf=wpf.astype(f8np if USE_FP8_PROJ else bf16),
        id64=(np.eye(128, dtype=f32) * PSCALE).astype(bf16),
    )
    if use_const:
        params["cvec"] = const.reshape(1, 2, 512).astype(bf16)
    return params, use_const


def kernel(**inputs):
    x = np.asarray(inputs["x"], dtype=np.float32)
    ln_g = np.asarray(inputs["ln_g"], dtype=np.float32)
    ln_b = np.asarray(inputs["ln_b"], dtype=np.float32)

    fast = (np.allclose(ln_g, 1.0, atol=1e-12) and
            np.allclose(ln_b, 0.0, atol=1e-12))
    if not fast:
        return _np_reference(
            x, *[np.asarray(inputs[k], dtype=np.float32) for k in
                 ("anchors", "ln_g", "ln_b", "W1", "b1", "W2", "b2", "cg",
                  "cb", "Wp", "bp", "gate")])

    params, use_const = _pack_params(
        inputs["anchors"], ln_g, inputs["W1"], inputs["b1"], inputs["W2"],
        inputs["b2"], inputs["cg"], inputs["cb"], inputs["Wp"], inputs["bp"],
        inputs["gate"], interleaved_t=INTERLEAVED_T)

    nc = _build_program(S, use_const, INTERLEAVED_T, USE_RECIP_APPROX)

    from concourse.bass_utils import run_bass_kernel_spmd
    in_maps = []
    for b in range(NCORES):
        m = dict(params)
        m["x"] = np.ascontiguousarray(x[b])
        in_maps.append(m)
    res = run_bass_kernel_spmd(nc, in_maps, core_ids=list(range(NCORES)))
    out = np.stack([res.results[b]["out"] for b in range(NCORES)], axis=0)
    return out.reshape(B, S, D).astype(np.float32)


INTERLEAVED_T = True
USE_RECIP_APPROX = True
USE_FP8_PROJ = True
USE_ID_RESID = True


# revision 17
# speedup vs baseline: 1.2062x; 1.1066x over previous
"""Trainium2 Bass kernel for nn_ConstellationRelay.

Computation (per token, D=1024, A=16 anchors, C=8 comps, dc=64):
  h   = l2norm(layernorm(x; ln_g, ln_b))
  tri = 1 - h @ l2norm(anchors).T                       (N, 16)
  u   = relu(einsum('nak,kae->nke', tri_g, W1) + b1)^2  (N, 8, 128)
  y   = layernorm_c(u @ W2 + b2; cg, cb)                (N, 8, 64)
  out = x + sigmoid(gate) * (y.flat @ Wp + bp)

Strategy: pure data-parallel over batch (one of 8 NeuronCores per batch row).
v3 design:
  * x cast f32->bf16 during the HBM load DMA (SWDGE, one op per tile);
    out written bf16->f32 by the store DMA. No f32 x on chip.
  * stats via tensor_scalar/tensor_tensor accum_out (sum, sumsq) on DVE.
  * residual folded into the proj PSUM group via a 64*I bf16 identity matmul
    (wpf prescaled 2^6 for fp8; drain copies apply 2^-6 -- exact).
  * proj matmul fp8 DoubleRow (contract 2x128 per MM).
  * issue order per round: dma_in(t), back(t-3), mid(t-2), front(t-1) --
    oldest work first in each engine FIFO to avoid head-of-line blocking.
"""

import functools
import os
import sys

import numpy as np

for _p in ("/opt/trn_rl_repo",):
    if _p not in sys.path and os.path.isdir(_p):
        sys.path.insert(0, _p)

B, S, D = 8, 4096, 1024
A, C, DC = 16, 8, 64
APC = A // C  # anchors per compartment
E2 = 2 * DC  # 128, expanded width per comp
NCORES = 8
TOK = 512  # tokens per pipeline tile
NTILE = S // TOK  # 8
NCH = TOK // 128  # 4 token chunks of 128 per tile
KD = D // 128  # 8 feature chunks
PSCALE = 64.0  # 2^6 fp8 pre-scale on wpf (and on the identity residual)


def _np_reference(x, anchors, ln_g, ln_b, W1, b1, W2, b2, cg, cb, Wp, bp, gate):
    x = x.astype(np.float32)
    N = x.shape[0] * x.shape[1]
    xf = x.reshape(N, D)
    mu = xf.mean(-1, keepdims=True)
    var = ((xf - mu) ** 2).mean(-1, keepdims=True)
    h = (xf - mu) / np.sqrt(var + 1e-5) * ln_g + ln_b
    h = h / np.maximum(np.linalg.norm(h, axis=-1, keepdims=True), 1e-12)
    a = anchors / np.maximum(np.linalg.norm(anchors, axis=-1, keepdims=True), 1e-12)
    tri = 1.0 - h @ a.T
    g = tri.reshape(N, APC, C)
    u = np.einsum("nak,kae->nke", g, W1) + b1
    u = np.square(np.maximum(u, 0.0))
    y = np.einsum("nke,ked->nkd", u, W2) + b2
    muy = y.mean(-1, keepdims=True)
    vy = ((y - muy) ** 2).mean(-1, keepdims=True)
    y = (y - muy) / np.sqrt(vy + 1e-5) * cg + cb
    upd = y.reshape(N, C * DC) @ Wp + bp
    sig = 1.0 / (1.0 + np.exp(-gate))
    return (xf + sig * upd).reshape(x.shape).astype(np.float32)


@functools.lru_cache(maxsize=4)
def _build_program(n_tokens=S, use_const=False, interleaved_t=True,
                   use_recip_approx=True, use_fp8=None, use_id=None):
    use_fp8 = USE_FP8_PROJ if use_fp8 is None else use_fp8
    use_id = USE_ID_RESID if use_id is None else use_id
    import concourse.bacc as bacc
    import concourse.mybir as mybir
    import concourse.tile as tile

    f32 = mybir.dt.float32
    bf16 = mybir.dt.bfloat16
    f8 = mybir.dt.float8e4
    AF = mybir.ActivationFunctionType
    OP = mybir.AluOpType
    PM = mybir.MatmulPerfMode

    ntile = n_tokens // TOK

    nc = bacc.Bacc("TRN2", target_bir_lowering=False, debug=False,
                   num_devices=NCORES)

    x_d = nc.dram_tensor("x", [n_tokens, D], f32, kind="ExternalInput")
    agt_d = nc.dram_tensor("agt", [128, KD, 112], bf16, kind="ExternalInput")
    w1e_d = nc.dram_tensor("w1e", [112, KD, 128], bf16, kind="ExternalInput")
    biasu_d = nc.dram_tensor("biasu", [128, KD], f32, kind="ExternalInput")
    w2c_d = nc.dram_tensor("w2c", [128, C, DC], bf16, kind="ExternalInput")
    vstl_d = nc.dram_tensor("vstl", [128, 4, C], bf16, kind="ExternalInput")
    b2f_d = nc.dram_tensor("b2f", [128, 4], f32, kind="ExternalInput")
    if use_fp8:
        wpf_d = nc.dram_tensor("wpf", [128, 2, 2, 2, 512], f8,
                               kind="ExternalInput")
    else:
        wpf_d = nc.dram_tensor("wpf", [128, 4, 2, 512], bf16,
                               kind="ExternalInput")
    sel_d = nc.dram_tensor("sel", [C, 4, 128], bf16, kind="ExternalInput")
    id64_d = nc.dram_tensor("id64", [128, 128], bf16, kind="ExternalInput")
    cvec_d = nc.dram_tensor("cvec", [1, 2, 512], bf16, kind="ExternalInput") \
        if use_const else None
    out_d = nc.dram_tensor("out", [n_tokens, D], f32, kind="ExternalOutput")

    from contextlib import ExitStack

    with tile.TileContext(nc) as tc, ExitStack() as ctx:
        pp = ctx.enter_context(tc.tile_pool(name="params", bufs=1))
        agt = pp.tile([128, KD, 112], bf16)
        nc.sync.dma_start(out=agt, in_=agt_d[:, :, :])
        w1e = pp.tile([112, KD, 128], bf16)
        nc.sync.dma_start(out=w1e, in_=w1e_d[:, :, :])
        biasu = pp.tile([128, KD], f32)
        nc.sync.dma_start(out=biasu, in_=biasu_d[:, :])
        w2c = pp.tile([128, C, DC], bf16)
        nc.sync.dma_start(out=w2c, in_=w2c_d[:, :, :])
        vstl = pp.tile([128, 4, C], bf16)
        nc.sync.dma_start(out=vstl, in_=vstl_d[:, :, :])
        b2f = pp.tile([128, 4], f32)
        nc.sync.dma_start(out=b2f, in_=b2f_d[:, :])
        if use_fp8:
            wpf = pp.tile([128, 2, 2, 2, 512], f8)
            nc.sync.dma_start(out=wpf, in_=wpf_d[:, :, :, :, :])
        else:
            wpf = pp.tile([128, 4, 2, 512], bf16)
            nc.sync.dma_start(out=wpf, in_=wpf_d[:, :, :, :])
        sel = pp.tile([C, 4, 128], bf16)
        nc.sync.dma_start(out=sel, in_=sel_d[:, :, :])
        id64 = pp.tile([128, 128], bf16)
        nc.sync.dma_start(out=id64, in_=id64_d[:, :])
        if use_const:
            cvec = pp.tile([1, 2, 512], bf16)
            nc.sync.dma_start(out=cvec, in_=cvec_d[:, :, :])
            ones1 = pp.tile([1, 128], bf16)
            nc.vector.memset(ones1, 1.0)
        ctiny = pp.tile([128, 1], f32)
        nc.vector.memset(ctiny, 1e-38)
        ceps = pp.tile([C, 1], f32)
        nc.vector.memset(ceps, 1e-5)

        px = ctx.enter_context(tc.tile_pool(name="px", bufs=2))
        psm = ctx.enter_context(tc.tile_pool(name="psm", bufs=8))
        # PSUM pools: 4 + 2 + 2 = 8 banks exactly.
        ps_small = ctx.enter_context(tc.tile_pool(name="ps_small", bufs=2,
                                                  space="PSUM"))
        ps_y = ctx.enter_context(tc.tile_pool(name="ps_y", bufs=2,
                                              space="PSUM"))
        ps_mm = ctx.enter_context(tc.tile_pool(name="ps_mm", bufs=4,
                                               space="PSUM"))

        def stage_load(t):
            row0 = t * TOK
            xb = px.tile([128, NCH, D], bf16, tag="xb", bufs=5, name=f"xb{t}")
            nc.gpsimd.dma_start(
                out=xb,
                in_=x_d[row0:row0 + TOK, :].rearrange(
                    "(c p) d -> p c d", p=128))
            return xb

        def stage_front(t, xb):
            """Stats + normalize + transpose."""
            mv = psm.tile([128, NCH, 2], f32, tag="mv", name=f"mv{t}")
            for cch in range(NCH):
                st = psm.tile([128, 2, 6], f32, tag="st")
                xr = xb[:, cch, :].rearrange("p (s f) -> p s f", s=2)
                nc.vector.bn_stats(out=st[:, 0, :], in_=xr[:, 0, :])
                nc.vector.bn_stats(out=st[:, 1, :], in_=xr[:, 1, :])
                nc.vector.bn_aggr(out=mv[:, cch, :], in_=st)
            sd = psm.tile([128, NCH], f32, tag="sd")
            nc.scalar.activation(sd, mv[:, :, 1], AF.Sqrt, bias=ctiny,
                                 scale=float(D))
            ee = psm.tile([128, NCH], f32, tag="ee", name=f"ee{t}")
            nc.vector.reciprocal(ee, sd)
            bh = psm.tile([128, NCH], f32, tag="bh", name=f"bh{t}")
            nc.vector.scalar_tensor_tensor(
                out=bh, in0=mv[:, :, 0], scalar=-1.0, in1=ee,
                op0=OP.mult, op1=OP.mult)
            hb = px.tile([128, NCH, D], bf16, tag="hb", bufs=3, name=f"hb{t}")
            for cch in range(NCH):
                nc.vector.tensor_scalar(
                    out=hb[:, cch, :], in0=xb[:, cch, :],
                    scalar1=ee[:, cch:cch + 1], scalar2=bh[:, cch:cch + 1],
                    op0=OP.mult, op1=OP.add)
            hbT = px.tile([128, KD, TOK], bf16, tag="hbT", bufs=2,
                          name=f"hbT{t}")
            for cch in range(NCH):
                nc.sync.dma_start_transpose(
                    out=hbT[:, :, cch * 128:(cch + 1) * 128],
                    in_=hb[:, cch, :])
            return hbT

        def stage_mid(t, xb, hbT):
            # --- A0 = a_norm @ h, 4 replicas at partitions {0,32,64,96} ---
            a0p = ps_small.tile([112, TOK], f32, tag="small")
            for dch in range(KD):
                nc.tensor.matmul(a0p, lhsT=agt[:, dch, :], rhs=hbT[:, dch, :],
                                 start=(dch == 0), stop=(dch == KD - 1))
            a0 = psm.tile([112, TOK], bf16, tag="a0", bufs=3)
            nc.scalar.copy(out=a0, in_=a0p)

            # --- expand (4-way row-packed) + relu; square on GPS ----------
            rbig = px.tile([128, KD, TOK], bf16, tag="rbig", bufs=3)
            ubig = px.tile([128, KD, TOK], bf16, tag="ubig", bufs=3)
            for kg in range(2):
                ups = []
                for r in range(4):
                    k = 4 * kg + r
                    up = ps_mm.tile([128, TOK], f32, tag="mmout")
                    nc.tensor.matmul(
                        up, lhsT=w1e[32 * r:32 * r + A, k, :],
                        rhs=a0[32 * r:32 * r + A, :],
                        start=True, stop=True,
                        tile_position=(32 * r, 0))
                    ups.append(up)
                for r in range(4):
                    k = 4 * kg + r
                    nc.scalar.activation(rbig[:, k, :], ups[r], AF.Relu,
                                         bias=biasu[:, k:k + 1], scale=1.0)
                    nc.gpsimd.tensor_mul(ubig[:, k, :], rbig[:, k, :],
                                         rbig[:, k, :])

            # --- comp matmul; yb via DVE ts, sqy via ACT Square -----------
            yb = px.tile([128, 4, TOK], bf16, tag="yb", bufs=3,
                         name=f"yb{t}")
            sqy = px.tile([128, 4, TOK], bf16, tag="sqy", bufs=3)
            for j in range(4):
                yp = ps_y.tile([128, TOK], f32, tag="ypre")
                nc.tensor.matmul(yp[0:64, :], lhsT=w2c[:, 2 * j, :],
                                 rhs=ubig[:, 2 * j, :], start=True, stop=True)
                nc.tensor.matmul(yp[64:128, :], lhsT=w2c[:, 2 * j + 1, :],
                                 rhs=ubig[:, 2 * j + 1, :], start=True,
                                 stop=True, tile_position=(0, 64))
                nc.scalar.activation(yb[:, j, :], yp, AF.Identity,
                                     bias=b2f[:, j:j + 1], scale=1.0)
                nc.gpsimd.tensor_mul(sqy[:, j, :], yb[:, j, :],
                                     yb[:, j, :])

            # --- per-comp variance via PE; rstd = 1/sqrt(var+eps) ---------
            vst = ps_small.tile([C, TOK], f32, tag="small")
            for j in range(4):
                nc.tensor.matmul(vst, lhsT=vstl[:, j, :], rhs=sqy[:, j, :],
                                 start=(j == 0), stop=(j == 3))
            sd2 = psm.tile([C, TOK], f32, tag="sd2", bufs=2)
            nc.scalar.activation(sd2, vst, AF.Sqrt, bias=ceps, scale=1.0)
            rr = psm.tile([C, TOK], f32, tag="rr", bufs=2)
            if use_recip_approx:
                nc.vector.reciprocal_approx_fast(out=rr, in_=sd2)
            else:
                nc.vector.reciprocal(out=rr, in_=sd2)
            rrb = psm.tile([C, TOK], bf16, tag="rrb", bufs=3, name=f"rrb{t}")
            nc.vector.tensor_copy(out=rrb, in_=rr)
            return yb, rrb

        def stage_back(t, xb, yb, rrb):
            row0 = t * TOK
            # rstd broadcast via selector matmuls; ycT = yb * rstd (fp8)
            ycT = px.tile([128, 4, TOK], f8 if use_fp8 else bf16,
                          tag="ycT", bufs=2)
            for j in range(4):
                rbP = ps_mm.tile([128, TOK], f32, tag="mmout")
                nc.tensor.matmul(rbP, lhsT=sel[:, j, :], rhs=rrb,
                                 start=True, stop=True)
                nc.vector.tensor_mul(ycT[:, j, :], yb[:, j, :], rbP)

            # --- proj (fp8 DoubleRow) + identity residual in PSUM ---------
            for cch in range(NCH):
                osb = px.tile([128, D], f32, tag="osb", bufs=6)
                for hf in range(2):
                    ud = ps_mm.tile([128, 512], f32, tag="mmout")
                    # last matmul of the accumulation group carries stop=True
                    last_is_proj = not (use_const or use_id)
                    if use_fp8:
                        for g in range(2):
                            nc.tensor.matmul(
                                ud,
                                lhsT=ycT[:, 2 * g:2 * g + 2,
                                         cch * 128:(cch + 1) * 128],
                                rhs=wpf[:, g, :, hf, :],
                                start=(g == 0),
                                stop=(last_is_proj and g == 1),
                                perf_mode=PM.DoubleRow)
                    else:
                        for j in range(4):
                            nc.tensor.matmul(
                                ud,
                                lhsT=ycT[:, j, cch * 128:(cch + 1) * 128],
                                rhs=wpf[:, j, hf, :],
                                start=(j == 0),
                                stop=(last_is_proj and j == 3))
                    if use_const:
                        nc.tensor.matmul(ud, lhsT=ones1, rhs=cvec[:, hf, :],
                                         start=False, stop=not use_id)
                    dst = osb[:, hf * 512:(hf + 1) * 512]
                    if use_id:
                        nc.tensor.matmul(
                            ud, lhsT=id64,
                            rhs=xb[:, cch, hf * 512:(hf + 1) * 512],
                            start=False, stop=True)
                        if hf == 0:
                            nc.vector.tensor_scalar(
                                out=dst, in0=ud, scalar1=1.0 / PSCALE,
                                scalar2=None, op0=OP.mult)
                        else:
                            nc.scalar.activation(dst, ud, AF.Copy,
                                                 scale=1.0 / PSCALE)
                    else:
                        nc.vector.scalar_tensor_tensor(
                            out=dst, in0=ud, scalar=1.0 / PSCALE,
                            in1=xb[:, cch, hf * 512:(hf + 1) * 512],
                            op0=OP.mult, op1=OP.add)
                nc.sync.dma_start(
                    out=out_d[row0 + cch * 128: row0 + (cch + 1) * 128, :],
                    in_=osb)

        # Software pipeline: load(t) | front(t-1) | mid(t-2) | back(t-3).
        # Issue order within a round: prefetch DMA first, then oldest work.
        xbs, hbTs, mds = {}, {}, {}
        for t in range(ntile + 3):
            if t < ntile:
                xbs[t] = stage_load(t)
            if t >= 3:
                yb_, rrb_ = mds.pop(t - 3)
                stage_back(t - 3, xbs.pop(t - 3), yb_, rrb_)
            if 2 <= t <= ntile + 1:
                mds[t - 2] = stage_mid(t - 2, xbs[t - 2], hbTs.pop(t - 2))
            if 1 <= t <= ntile:
                hbTs[t - 1] = stage_front(t - 1, xbs[t - 1])

    nc.compile()
    return nc


def _pack_params(anchors, ln_g, W1, b1, W2, b2, cg, cb, Wp, bp, gate,
                 interleaved_t=True):
    f32 = np.float32
    anchors = anchors.astype(f32)
    an = anchors / np.maximum(
        np.linalg.norm(anchors.astype(np.float64), axis=1, keepdims=True),
        1e-12).astype(f32)
    ag = (an * ln_g[None, :].astype(f32)).astype(f32)  # [A, D]

    # agt[p, s, 32r+m] = ag[m, d(p,s)] for r in 0..3 (4 replicas)
    agt = np.zeros((128, KD, 112), f32)
    dd = np.arange(D)
    if interleaved_t:
        pidx, sidx = dd // KD, dd % KD
    else:
        pidx, sidx = dd % 128, dd // 128
    for r in range(4):
        agt[pidx, sidx, 32 * r:32 * r + A] = ag.T[dd, :]

    # W1exp[m, f] with m=j*C+k2, f=k*128+e -> value W1[k, j, e] iff k2==k
    W1 = W1.astype(f32)
    w1exp = np.zeros((A, C, E2), f32)
    for m in range(A):
        j, k2 = m // C, m % C
        w1exp[m, k2, :] = W1[k2, j, :]
    w1e16 = (-w1exp).reshape(A, C, E2)  # [16, 8, 128] (f = k*128+e)
    w1e = np.zeros((112, C, E2), f32)
    for r in range(4):
        w1e[32 * r:32 * r + A] = w1e16
    sf = w1exp.sum(axis=0)  # [C, E2]
    biasu = (sf + b1.astype(f32)).T.copy()  # [128, C] (partition=e, col=k)

    W2 = W2.astype(f32)
    w2m = W2.mean(axis=2, keepdims=True)
    w2cent = W2 - w2m  # [C, E2, DC]
    w2c = np.transpose(w2cent, (1, 0, 2)).copy()  # [128, C, 64]
    b2c = b2.astype(f32) - b2.astype(f32).mean(axis=1, keepdims=True)  # [C, DC]

    b2f = np.zeros((128, 4), f32)
    vstl = np.zeros((128, 4, C), f32)
    for j in range(4):
        for p in range(128):
            kk = 2 * j + p // 64
            b2f[p, j] = b2c[kk, p % 64]
            vstl[p, j, kk] = 1.0 / DC

    sig = (1.0 / (1.0 + np.exp(-gate.astype(np.float64)))).astype(f32)  # [D]
    wpfold = (cg.astype(f32).reshape(C * DC, 1) * Wp.astype(f32)) * sig[None, :]
    wpfold = wpfold * PSCALE
    if USE_FP8_PROJ:
        # DoubleRow: wpf[p, g, i, hf, f] = wpfold[(2g+i)*128 + p, hf*512+f]
        wpf = np.ascontiguousarray(
            wpfold.reshape(2, 2, 128, 2, 512).transpose(2, 0, 1, 3, 4))
    else:
        wpf = np.ascontiguousarray(
            wpfold.reshape(4, 128, 2, 512).transpose(1, 0, 2, 3))

    const = (cb.astype(f32).reshape(-1) @ Wp.astype(f32) + bp.astype(f32)) * sig
    use_const = bool(np.max(np.abs(const)) > 0)

    import ml_dtypes
    bf16 = ml_dtypes.bfloat16
    import concourse.mybir as mybir
    f8np = mybir.dt.np(mybir.dt.float8e4)
    sel = np.zeros((C, 4, 128), f32)
    for j in range(4):
        sel[2 * j, j, 0:64] = 1.0
        sel[2 * j + 1, j, 64:128] = 1.0

    params = dict(
        sel=sel.astype(bf16),
        agt=agt.astype(bf16),
        w1e=w1e.astype(bf16),
        biasu=biasu.astype(f32),
        w2c=w2c.astype(bf16),
        vstl=vstl.astype(bf16),
        b2f=b2f.astype(f32),
        wpf=wpf.astype(f8np if USE_FP8_PROJ else bf16),
        id64=(np.eye(128, dtype=f32) * PSCALE).astype(bf16),
    )
    if use_const:
        params["cvec"] = const.reshape(1, 2, 512).astype(bf16)
    return params, use_const


def kernel(**inputs):
    x = np.asarray(inputs["x"], dtype=np.float32)
    ln_g = np.asarray(inputs["ln_g"], dtype=np.float32)
    ln_b = np.asarray(inputs["ln_b"], dtype=np.float32)

    fast = (np.allclose(ln_g, 1.0, atol=1e-12) and
            np.allclose(ln_b, 0.0, atol=1e-12))
    if not fast:
        return _np_reference(
            x, *[np.asarray(inputs[k], dtype=np.float32) for k in
                 ("anchors", "ln_g", "ln_b", "W1", "b1", "W2", "b2", "cg",
                  "cb", "Wp", "bp", "gate")])

    params, use_const = _pack_params(
        inputs["anchors"], ln_g, inputs["W1"], inputs["b1"], inputs["W2"],
        inputs["b2"], inputs["cg"], inputs["cb"], inputs["Wp"], inputs["bp"],
        inputs["gate"], interleaved_t=INTERLEAVED_T)

    nc = _build_program(S, use_const, INTERLEAVED_T, USE_RECIP_APPROX)

    from concourse.bass_utils import run_bass_kernel_spmd
    in_maps = []
    for b in range(NCORES):
        m = dict(params)
        m["x"] = np.ascontiguousarray(x[b])
        in_maps.append(m)
    res = run_bass_kernel_spmd(nc, in_maps, core_ids=list(range(NCORES)))
    out = np.stack([res.results[b]["out"] for b in range(NCORES)], axis=0)
    return out.reshape(B, S, D).astype(np.float32)


INTERLEAVED_T = True
USE_RECIP_APPROX = True
USE_FP8_PROJ = True
USE_ID_RESID = True
